# revision 69
# baseline (speedup 1.0000x reference)
"""Trainium2 Bass kernel for BiasFreeDenoisingGNN (N=1024, H=128, E=32768, L=3).

Strategy (8 NeuronCores, one SPMD program, NO collectives):
  - Message passing is fully REPLICATED on every core (collectives cost
    >=15us each, so sharded+AllGather designs lose).  The segment-sum is
    aggT = sum_c relu_r1_c^T @ adj_c with adjacency edge-counts in fp8e4
    (exact small ints, DoubleRow perf mode); the mean-division is fused
    into the PSUM->SBUF copy as a multiply with a broadcast 1/deg row
    (broadcast on-device via gpsimd partition_broadcast from a [1,N] DMA).
  - All f32/bf16 inputs ride in ONE packed f32 dram tensor ("main") with
    bf16 regions read through bitcast views; DMAs are split in need-order
    (h0/weights first, adjacency quarters, remaining weights last) to
    minimize the per-DMA HWDGE/SEQ ladder (each DMA adds a fixed ~1.3us
    issue-latency step).
  - PSUM tiles are split per-half everywhere two engines evict or a write
    follows a disjoint read: the tile dep-tracker serializes cross-engine
    accesses of one PSUM tile (~0.6us per occurrence); p_h, p_r1, p_up,
    p_a, p_b are all split.  HW NOTE: emitting a chunk's basis and rlp
    matmuls as two separate passes corrupts results on hardware (sim is
    fine) - keep each chunk's start/stop accumulation pair adjacent.
  - Edge predictor: 512 virtual rows of 1024 pairs across 8 cores, rows
    v=8t+k fused with 1022-v so each VR is one dense 1024-slot span.
    X = relu(A_i + B_j) built by Pool(POOL_X-col static slice)+DVE
    tensor_scalars at DVE 4x; eW2 on PE; the relu PSUM->SBUF transport is
    batched over 1.5-VR (1536-col) batches into an Act-evicted [P,1024]
    tile plus a DVE-evicted [P,512] tile (separate tags -> true double
    buffering, one reader per PSUM tile, amortized access penalties);
    eW3 = 8 tiny 2-col matmuls per VR into persistent PSUM staging copied
    out by Act once per 16 VRs.  Tiny keep-warm matmuls tied to prep
    outputs hold the PE p-state up through the MP->predictor gap.
"""
import sys
import numpy as np

sys.path.insert(0, "/opt/trn_rl_repo")

import concourse.bass as bass  # noqa: E402
import concourse.bacc as bacc  # noqa: E402
import concourse.mybir as mybir  # noqa: E402
import concourse.tile as tile  # noqa: E402
from concourse.bass_utils import run_bass_kernel_spmd  # noqa: E402
import ml_dtypes  # noqa: E402

N = 1024
H = 128
E = 32768
L = 3
C = 10
NCORES = 8
P = 128
VR = 64             # virtual rows per core
DT = mybir.dt
F32 = DT.float32
F32R = DT.float32r
BF16 = DT.bfloat16
FP8 = DT.float8e4
AF = mybir.ActivationFunctionType
OP = mybir.AluOpType

# ---- packed "main" f32 input layout (f32 cols) ----
# [wf32 130 | proj(bf16) 64 | h0t(bf16) 512 | wbf(bf16) 769]
WF32_O, WF32_C = 0, 130
PROJ_O = 130            # bf16 [128, 128] as 64 f32 cols
H0T_O = 194             # bf16 [128, 1024] as 512 f32 cols
WBF_O = 706             # bf16 [128, 1538] as 769 f32 cols
MAIN_COLS = 1475
# bf16 col offsets inside the wbf view (layer-major)
WL = 3 * H              # per-layer stride: m1 | upd | w2u
E1TO, E1BO, E2O, E3O = 3 * WL, 3 * WL + H, 3 * WL + 2 * H, 3 * WL + 3 * H
WBF_COLS = 3 * WL + 3 * H + 2
# DMA split points (f32 cols of main): D0 = [0:770] (wf32..m1L0),
# D3 = [770:898] (updL0,w2uL0), D5 = [898:1475] (L1,L2,e-weights)
D0_END = WBF_O + (WL + H) // 2      # through m1L0: 706 + 64 = 770... (m1L0 is wbf[0:128])
D0_END = WBF_O + H // 2 * 1         # 706 + 64 = 770
D3_END = WBF_O + (H + 2 * H) // 2   # through w2uL0: 706 + 192 = 898

import os as _os
POOL_X = int(_os.environ.get("KPOOLX", 426))   # Pool's fixed X-build slice per VR
DVE_R = int(_os.environ.get("KDVER", 328))     # DVE's relu slice per VR

_CACHE = {}
LAST_RESULTS = None
TRACE = False


def _build_nc(phases="all"):
    import os
    phases = os.environ.get("KPHASES", phases)
    nc = bacc.Bacc("TRN2", target_bir_lowering=False, debug=False,
                   enable_asserts=True, num_devices=NCORES)
    # --- kernel I/O ---
    main_d = nc.dram_tensor("main", [P, MAIN_COLS], F32, kind="ExternalInput")
    adj_d = nc.dram_tensor("adj8", [P, 8 * N], FP8, kind="ExternalInput")
    rdeg_d = nc.dram_tensor("rdeg", [1, N], F32, kind="ExternalInput")
    logits_d = nc.dram_tensor("logits_v", [P, 1024], F32, kind="ExternalOutput")
    debug = os.environ.get("KDEBUG") == "1"
    if debug:
        dbg_rsc = nc.dram_tensor("dbg_rsc", [P, N], BF16, kind="ExternalOutput")
        dbg_rls = nc.dram_tensor("dbg_rls", [P, N], BF16, kind="ExternalOutput")
        dbg_rdg = nc.dram_tensor("dbg_rdg", [P, N], F32, kind="ExternalOutput")
        dbg_r1 = nc.dram_tensor("dbg_r1", [P, N], BF16, kind="ExternalOutput")
        dbg_h1 = nc.dram_tensor("dbg_h1", [P, N], BF16, kind="ExternalOutput")
        dbg_hf = nc.dram_tensor("dbg_hf", [P, N], F32, kind="ExternalOutput")
        dbg_at = nc.dram_tensor("dbg_at", [P, N], F32, kind="ExternalOutput")
        dbg_bx = nc.dram_tensor("dbg_bx", [P, 2 * N], BF16, kind="ExternalOutput")
        dbg_xb = nc.dram_tensor("dbg_xb", [P, 1032], BF16, kind="ExternalOutput")
        dbg_rb = nc.dram_tensor("dbg_rb", [P, N], BF16, kind="ExternalOutput")

    with tile.TileContext(nc) as tc:
        with tc.tile_pool(name="cst", bufs=1) as cst, \
             tc.tile_pool(name="wk", bufs=3) as wk, \
             tc.tile_pool(name="xp", bufs=4) as xp, \
             tc.tile_pool(name="rp", bufs=5) as rp, \
             tc.tile_pool(name="ps", bufs=2, space="PSUM") as ps:

            kreg = nc.partition_id()

            # ---- constant loads (order = need order) ----
            main = cst.tile([P, MAIN_COLS], F32)
            adj8 = cst.tile([P, 8 * N], FP8)
            rdeg1 = cst.tile([1, N], F32)
            rdegb = cst.tile([P, N], F32)
            nc.sync.dma_start(main[:, 0:D0_END], main_d[:, 0:D0_END])
            nc.sync.dma_start(rdeg1[:], rdeg_d[:])
            nc.sync.dma_start(adj8[:, 0:2 * N], adj_d[:, 0:2 * N])
            nc.sync.dma_start(adj8[:, 2 * N:4 * N], adj_d[:, 2 * N:4 * N])
            nc.sync.dma_start(adj8[:, 4 * N:6 * N], adj_d[:, 4 * N:6 * N])
            nc.sync.dma_start(adj8[:, 6 * N:8 * N], adj_d[:, 6 * N:8 * N])
            nc.sync.dma_start(main[:, D0_END:D3_END], main_d[:, D0_END:D3_END])
            nc.sync.dma_start(main[:, D3_END:MAIN_COLS],
                              main_d[:, D3_END:MAIN_COLS])
            nc.gpsimd.partition_broadcast(rdegb[:], rdeg1[:])
            wf32 = main[:, WF32_O:WF32_O + WF32_C]
            proj_t = main[:, PROJ_O:H0T_O].bitcast(BF16)
            h0t = main[:, H0T_O:WBF_O].bitcast(BF16)
            wbf = main[:, WBF_O:MAIN_COLS].bitcast(BF16)

            # warm-ups while DMAs run: hoist the 1.3us ACT_TABLE_LOAD off the
            # h0 chain, and start the PE p-state ramp clock early with a tiny
            # matmul on memset data; the ones row also broadcasts rdeg via PE
            warm = cst.tile([P, 1], BF16)
            nc.vector.memset(warm[:], 0.0)
            warm2 = cst.tile([P, 1], F32)
            nc.scalar.activation(warm2[:], warm[:], AF.Relu)
            p_warm = ps.tile([P, 1], F32, space="PSUM", tag="half")
            nc.tensor.matmul(out=p_warm[0:1, 0:1], lhsT=warm[:, 0:1],
                             rhs=warm[:, 0:1], start=True, stop=True)

            t_rep = wf32[:, 0:1]
            tw1t = wf32[:, 1:2]
            w2p = wf32[:, 2:2 + P]

            # ---- h0: x1 = relu(t * tW1^T); t2 = (tW2@projW)^T @ x1 ----
            x1t = cst.tile([H, 1], F32)
            nc.vector.tensor_scalar(out=x1t[:], in0=tw1t, scalar1=t_rep,
                                    scalar2=0.0, op0=OP.mult, op1=OP.max)
            p_t2 = ps.tile([P, 1], F32, space="PSUM", tag="half")
            nc.tensor.matmul(out=p_t2[:], lhsT=w2p, rhs=x1t[:],
                             start=True, stop=True)
            t2 = cst.tile([P, 1], F32)
            nc.vector.tensor_copy(t2[:], p_t2[:])

            # h1 = relu(h0 @ projW + t2)  (t2 per-partition bias).  Per-half
            # PSUM tiles (coarse PSUM dep-tracking would serialize the second
            # mm behind the first half's relu otherwise); h1 lands in f32 as
            # h32[0] and hbf[0] is a cheap SBUF->SBUF downcast on DVE.
            h32 = [cst.tile([P, N], F32, name=f"h32_{i}") for i in range(2)]
            hbf = [cst.tile([P, N], BF16, name=f"hbf_{i}") for i in range(2)]
            p_hs = [ps.tile([P, 512], F32, space="PSUM", tag="half",
                            name=f"p_h{hh}") for hh in range(2)]
            for half in range(2):
                sl = slice(half * 512, (half + 1) * 512)
                nc.tensor.matmul(out=p_hs[half][:], lhsT=proj_t,
                                 rhs=h0t[:, sl], start=True, stop=True)
                if half == 0:
                    nc.vector.tensor_scalar(out=h32[0][:, sl], in0=p_hs[0][:],
                                            scalar1=t2[:, :1], scalar2=0.0,
                                            op0=OP.add, op1=OP.max)
                else:
                    nc.scalar.activation(h32[0][:, sl], p_hs[1][:], AF.Relu,
                                         bias=t2[:, :1])
                nc.vector.tensor_copy(hbf[0][:, sl], h32[0][:, sl])
            emit_h32_0 = []
            if debug:
                nc.sync.dma_start(dbg_h1[:], hbf[0][:])

            # ---- message passing (replicated, no comms) ----
            # msgW2 is host-folded into the update weights (W2U = msgW2 @
            # updW_bot), so each layer is: r1 rows -> relu -> adjacency
            # contraction -> deg-scale -> update.  Layer l>=1 reads h via the
            # linear basis (h_{l-1}, rl_{l-1}) so the residual adds run off
            # the critical path.
            rl_prev = None
            for l in (range(L) if phases in ("all", "mp") else []):
                cur, nxt = l % 2, (l + 1) % 2
                m1l = wbf[:, l * WL:l * WL + H]
                updl = wbf[:, l * WL + H:l * WL + 2 * H]
                w2ul = wbf[:, l * WL + 2 * H:l * WL + 3 * H]

                # r1 rows: chunk c -> [node, h1]; two per-half PSUM tiles so
                # the Act/DVE relu halves don't get a false cross-reader dep
                p_r1s = [ps.tile([P, 512], F32, space="PSUM", tag="half",
                                 name=f"p_r1{l}{hh}") for hh in range(2)]
                for c in range(8):
                    osl = slice(c * P, (c + 1) * P)
                    dst = p_r1s[c // 4][:, (c % 4) * P:(c % 4 + 1) * P]
                    if l == 0:
                        nc.tensor.matmul(out=dst, lhsT=hbf[cur][:, osl],
                                         rhs=m1l, start=True, stop=True)
                    else:
                        rlp = rl_prev[c // 4][:, (c % 4) * P:(c % 4 + 1) * P]
                        nc.tensor.matmul(out=dst, lhsT=hbf[nxt][:, osl],
                                         rhs=m1l, start=True, stop=False)
                        nc.tensor.matmul(out=dst, lhsT=rlp,
                                         rhs=m1l, start=False, stop=True)
                # two separate dest tiles (Act/DVE), fewer dependency hops
                r1rh = [wk.tile([P, 512], FP8, tag=f"r1h{qq}",
                                name=f"r1r{l}{qq}") for qq in range(2)]
                nc.scalar.activation(r1rh[0][:], p_r1s[0][:], AF.Relu)
                nc.vector.tensor_scalar(out=r1rh[1][:], in0=p_r1s[1][:],
                                        scalar1=0.0, scalar2=None, op0=OP.max)
                # deferred off-chain work from the previous layer (runs in
                # this layer's agg window, keeps it off DVE's critical path).
                # MUST be emitted before the upd-base matmuls below, which
                # read the hbf[cur] these adds produce; natural priority so
                # the scheduler keeps them AFTER this layer's r1 relus.
                for fn in emit_h32_0:
                    fn()
                emit_h32_0 = []
                # upd "base" matmuls don't depend on agg: emit early so PE
                # fills the relu/agg wait gaps.  Per-half tiles: coarse PSUM
                # dep-tracking would serialize the stop-mm of half 1 behind
                # the rls read of half 0 otherwise.
                p_ups = [ps.tile([P, 512], F32, space="PSUM", tag="big",
                                 bufs=2, name=f"p_up{l}{hh}") for hh in range(2)]
                for half in range(2):
                    sl = slice(half * 512, (half + 1) * 512)
                    nc.tensor.matmul(out=p_ups[half][:], lhsT=updl,
                                     rhs=hbf[cur][:, sl], start=True, stop=False)

                # ragg[h1, d] = sum_c relu_r1_c^T @ adj_c  (h0/h1 col-halves in
                # separate PSUM tiles; a-outer order so the a={0,1} matmuls
                # need only the Act relu half + the first adjacency DMA)
                p_aggs = [ps.tile([P, 512], F32, space="PSUM", tag="half",
                                  name=f"p_agg{l}{hh}") for hh in range(2)]
                # emission order: all of p_agg0's a-steps relu0-first, and
                # p_agg1's group STARTS at a=2 (needs the DVE relu anyway) so
                # its buffer-WAR wait on p_r1s[1] doesn't block the PE queue
                agg_order = [(0, 0), (1, 0), (2, 0), (3, 0),
                             (2, 1), (3, 1), (0, 1), (1, 1)]
                agg_start = {(0, 0): True, (2, 1): True}
                agg_stop = {(3, 0): True, (1, 1): True}
                for a, half in agg_order:
                    nc.tensor.matmul(
                        out=p_aggs[half][:],
                        lhsT=r1rh[a // 2][:, (a % 2) * 256:(a % 2 + 1) * 256]
                            .rearrange("p (c h) -> p c h", c=2),
                        rhs=adj8[:, 2048 * a + 1024 * half:
                                 2048 * a + 1024 * half + 1024]
                            .rearrange("p (c n) -> p c n", c=2),
                        start=agg_start.get((a, half), False),
                        stop=agg_stop.get((a, half), False),
                        perf_mode=mybir.MatmulPerfMode.DoubleRow)
                rscs = [wk.tile([P, 512], BF16, tag=f"agg{hh}",
                                name=f"rsc{l}{hh}") for hh in range(2)]
                rls = [wk.tile([P, 512], BF16, tag=f"rl{hh}",
                               name=f"rl{l}{hh}") for hh in range(2)]
                for half in range(2):
                    sl = slice(half * 512, (half + 1) * 512)
                    nc.vector.tensor_tensor(out=rscs[half][:], in0=p_aggs[half][:],
                                            in1=rdegb[:, sl], op=OP.mult)
                for half in range(2):
                    nc.tensor.matmul(out=p_ups[half][:], lhsT=w2ul,
                                     rhs=rscs[half][:], start=False, stop=True)
                    nc.scalar.activation(rls[half][:], p_ups[half][:], AF.Relu)
                if debug and l == 1:
                    nc.sync.dma_start(dbg_rsc[:, 0:512], rscs[0][:])
                    nc.sync.dma_start(dbg_rsc[:, 512:N], rscs[1][:])
                    nc.sync.dma_start(dbg_rls[:, 0:512], rls[0][:])
                    nc.sync.dma_start(dbg_rls[:, 512:N], rls[1][:])
                    nc.sync.dma_start(dbg_rdg[:], rdegb[:])
                    r1dbg = cst.tile([P, N], BF16)
                    nc.vector.tensor_copy(r1dbg[:, 0:512], r1rh[0][:])
                    nc.vector.tensor_copy(r1dbg[:, 512:N], r1rh[1][:])
                    nc.sync.dma_start(dbg_r1[:], r1dbg[:])
                if l < L - 1:  # residual adds, deferred into next layer's body
                    def mk_adds(rl_t, cu, nx):
                        def emit():
                            for half in range(2):
                                sl = slice(half * 512, (half + 1) * 512)
                                nc.vector.tensor_tensor(out=hbf[nx][:, sl],
                                                        in0=rl_t[half][:],
                                                        in1=h32[cu][:, sl],
                                                        op=OP.add)
                                nc.gpsimd.tensor_tensor(out=h32[nx][:, sl],
                                                        in0=rl_t[half][:],
                                                        in1=h32[cu][:, sl],
                                                        op=OP.add)
                        return emit
                    emit_h32_0 = [mk_adds(rls, cur, nxt)]
                rl_prev = rls

            # ---- predictor prep: A^T (f32), bext = [B^T, reversed B^T] ----
            # final h = hbf[cur-of-last-layer] + rl_prev (linear basis)
            fcur = (L - 1) % 2
            e1t = wbf[:, E1TO:E1TO + H]
            e1b = wbf[:, E1BO:E1BO + H]
            e2 = wbf[:, E2O:E2O + H]
            e3 = wbf[:, E3O:E3O + 2]
            if phases == "pred":
                fdelta = [hbf[0][:, 0:512], hbf[0][:, 512:N]]
            else:
                fdelta = [rl_prev[0][:], rl_prev[1][:]]
            fbase = hbf[fcur]
            # per-half PSUM tiles for A/B so the Act and DVE evictions of
            # the two halves don't pick up a false cross-reader dependency
            p_as = [ps.tile([P, 512], F32, space="PSUM", tag="dj", bufs=2,
                            name=f"p_a{hh}") for hh in range(2)]
            for half in range(2):
                sl = slice(half * 512, (half + 1) * 512)
                nc.tensor.matmul(out=p_as[half][:], lhsT=e1t, rhs=fbase[:, sl],
                                 start=True, stop=False)
                nc.tensor.matmul(out=p_as[half][:], lhsT=e1t, rhs=fdelta[half],
                                 start=False, stop=True)
            AT_f = cst.tile([P, N], F32)
            nc.scalar.activation(AT_f[:, 0:512], p_as[0][:], AF.Copy)
            nc.vector.tensor_copy(AT_f[:, 512:N], p_as[1][:])
            p_bs = [ps.tile([P, 512], F32, space="PSUM", tag="big", bufs=2,
                            name=f"p_b{hh}") for hh in range(2)]
            for half in range(2):
                sl = slice(half * 512, (half + 1) * 512)
                nc.tensor.matmul(out=p_bs[half][:], lhsT=e1b, rhs=fbase[:, sl],
                                 start=True, stop=False)
                nc.tensor.matmul(out=p_bs[half][:], lhsT=e1b, rhs=fdelta[half],
                                 start=False, stop=True)
            bext = cst.tile([P, 2 * N], BF16)
            nc.vector.tensor_copy(bext[:, 0:512], p_bs[0][:])
            nc.scalar.activation(bext[:, 512:N], p_bs[1][:], AF.Copy)
            # only rev cols [N, N+528) are ever read (wrap s2<=512, bsh tail);
            # build them with a 4x SBUF->SBUF reversed copy instead of a
            # full-width PSUM eviction
            nc.vector.tensor_copy(bext[:, N:N + 528],
                                  bext[:, 496:N][:, ::-1])
            # per-core shifted copies so the Pool X-build ops use fully
            # static access patterns (gpsimd mishandles register-offset APs);
            # ash + first bsh half unblock VR0's Pool op earliest
            ash = cst.tile([P, 512], F32)
            nc.scalar.activation(ash[:], AT_f[:, bass.ds(kreg, 512)], AF.Copy)
            # PE p-state keep-warm: tiny matmuls tied to prep outputs spread
            # across the otherwise-idle PE window so the first eW2 matmuls
            # don't pay the cold-ramp penalty
            for wi, wsrc in enumerate((bext[:, 0:1], bext[:, 700:701],
                                       bext[:, N:N + 1])):
                p_w = ps.tile([P, 1], F32, space="PSUM", tag="half",
                              name=f"p_wu{wi}")
                nc.tensor.matmul(out=p_w[0:1, 0:1], lhsT=wsrc,
                                 rhs=warm[:, 0:1], start=True, stop=True)
            bsh = cst.tile([P, 1032], BF16)
            nc.vector.tensor_copy(bsh[:, 0:516], bext[:, bass.ds(kreg, 516)])
            nc.vector.tensor_copy(bsh[:, 516:1032],
                                  bext[:, bass.ds(kreg + 516, 516)])
            if debug:
                nc.sync.dma_start(dbg_at[:], AT_f[:])
                nc.sync.dma_start(dbg_bx[:], bext[:])
                hfd = cst.tile([P, N], F32)
                for _h in range(2):
                    _sl = slice(_h * 512, (_h + 1) * 512)
                    nc.vector.tensor_tensor(out=hfd[:, _sl], in0=fbase[:, _sl],
                                            in1=fdelta[_h], op=OP.add)
                nc.sync.dma_start(dbg_hf[:], hfd[:])

            # ---- predictor: 64 virtual rows ----
            def emit_x(t):
                s1 = 1023 - 8 * t
                s2 = 8 * t + 8
                xb = xp.tile([P, 1032], BF16, tag="x")
                if t < 2:
                    # first VRs: DVE-only dynamic APs (proven on HW), so the
                    # Pool shift-prep (bsh/ash) finishes in their shadow
                    a_dyn = AT_f[:, bass.ds(kreg + 8 * t, 1)]
                    nc.vector.tensor_scalar(
                        out=xb[:, 0:s1],
                        in0=bext[:, bass.ds(kreg + (8 * t + 1), s1)],
                        scalar1=a_dyn, scalar2=0.0, op0=OP.add, op1=OP.max)
                else:
                    a_fwd = ash[:, 8 * t:8 * t + 1]
                    nc.gpsimd.tensor_scalar(
                        out=xb[:, 0:POOL_X],
                        in0=bsh[:, 8 * t + 1:8 * t + 1 + POOL_X],
                        scalar1=a_fwd, scalar2=0.0, op0=OP.add, op1=OP.max)
                    nc.vector.tensor_scalar(
                        out=xb[:, POOL_X:s1],
                        in0=bsh[:, 8 * t + 1 + POOL_X:8 * t + 1 + s1],
                        scalar1=a_fwd, scalar2=0.0, op0=OP.add, op1=OP.max)
                nc.vector.tensor_scalar(
                    out=xb[:, bass.ds(s1 - kreg, s2)],
                    in0=bext[:, N:N + s2],
                    scalar1=AT_f[:, bass.ds((1022 - 8 * t) - kreg, 1)],
                    scalar2=0.0, op0=OP.add, op1=OP.max)
                return xb

            # relu evictions are batched over 1.5-VR (1536-col) batches:
            # each batch owns an Act-evicted [P,1024] tile and a DVE-evicted
            # [P,512] tile (separate tags -> true double buffering, and one
            # reader per PSUM tile so no cross-engine reader serialization).
            # Amortizes the per-op PSUM access penalty over 1536 cols.
            NBCOLS = 1536
            TOT = VR * 1024
            NB = (TOT + NBCOLS - 1) // NBCOLS

            def emit_batch_mms(b, xbs):
                base = b * NBCOLS
                cols = min(NBCOLS, TOT - base)
                pa = ps.tile([P, 1024], F32, space="PSUM", tag="big",
                             bufs=2, name=f"pa{b}")
                pd = (ps.tile([P, 512], F32, space="PSUM", tag="dj",
                              bufs=2, name=f"pd{b}") if cols > 1024 else None)
                for c in range(0, cols, 512):
                    gc = base + c
                    v, half = gc // 1024, (gc % 1024) // 512
                    if v not in xbs:
                        xbs[v] = emit_x(v)
                        if debug and v == 0:
                            nc.sync.dma_start(dbg_xb[:], xbs[v][:])
                    tile_, off = (pa, c) if c < 1024 else (pd, c - 1024)
                    nc.tensor.matmul(out=tile_[:, off:off + 512], lhsT=e2,
                                     rhs=xbs[v][:, half * 512:half * 512 + 512],
                                     start=True, stop=True)
                for v in [k for k in xbs if (k + 1) * 1024 <= base + cols]:
                    del xbs[v]
                return (pa, pd, cols)

            def emit_batch_relu(b, pys_b):
                pa, pd, cols = pys_b
                ca = min(cols, 1024)
                rba = rp.tile([P, 1024], BF16, tag="ra", bufs=4,
                              name=f"rba{b}")
                nc.scalar.activation(rba[:, 0:ca], pa[:, 0:ca], AF.Relu)
                rbd = None
                if pd is not None:
                    rbd = rp.tile([P, 512], BF16, tag="rd", bufs=4,
                                  name=f"rbd{b}")
                    nc.vector.tensor_scalar(out=rbd[:, 0:cols - 1024],
                                            in0=pd[:, 0:cols - 1024],
                                            scalar1=0.0, scalar2=None,
                                            op0=OP.max)
                if debug and b == 0:
                    nc.sync.dma_start(dbg_rb[:], rba[:, 0:N])
                return (rba, rbd)

            def emit_ew3_v(v, rbs, p_os):
                blk = v // 32
                if v % 32 == 0:
                    p_os[blk] = ps.tile([P, 512], F32, space="PSUM",
                                        tag="half", name=f"p_o{blk}")
                w = v % 32
                for c in range(8):
                    gc = 1024 * v + c * P
                    bb, off = gc // NBCOLS, gc % NBCOLS
                    rba, rbd = rbs[bb]
                    rb_, o2 = (rba, off) if off < 1024 else (rbd, off - 1024)
                    nc.tensor.matmul(
                        out=p_os[blk][:, 16 * w + 2 * c:16 * w + 2 * c + 2],
                        lhsT=rb_[:, o2:o2 + P], rhs=e3,
                        start=True, stop=True)
                if v % 32 == 15:
                    stg = wk.tile([P, 512], F32, tag="stg", name=f"stg{blk}")
                    p_os[blk + 2] = stg
                    nc.scalar.activation(stg[:, 0:256], p_os[blk][:, 0:256],
                                         AF.Copy)
                    nc.sync.dma_start(logits_d[:, 512 * blk:512 * blk + 256],
                                      stg[:, 0:256])
                if blk == 1 and v % 32 == 23:
                    stg = p_os[3]
                    nc.scalar.activation(stg[:, 256:384], p_os[1][:, 256:384],
                                         AF.Copy)
                    nc.sync.dma_start(logits_d[:, 768:896], stg[:, 256:384])
                if v % 32 == 31:
                    stg = p_os[blk + 2]
                    lo = 384 if blk == 1 else 256
                    nc.scalar.activation(stg[:, lo:512], p_os[blk][:, lo:512],
                                         AF.Copy)
                    nc.sync.dma_start(
                        logits_d[:, 512 * blk + lo:512 * (blk + 1)],
                        stg[:, lo:512])

            if phases in ("all", "pred"):
                xbs = {}
                pys = {}
                rbs = {}
                p_os = {}
                done_v = 0
                for step in range(NB + 2):
                    if step < NB:
                        pys[step] = emit_batch_mms(step, xbs)
                    if 1 <= step <= NB:
                        rbs[step - 1] = emit_batch_relu(step - 1,
                                                        pys.pop(step - 1))
                    if step >= 2:
                        # all VRs fully covered by relu'd batches [0, step-1)
                        avail = ((step - 1) * NBCOLS) // 1024
                        while done_v < min(avail, VR):
                            emit_ew3_v(done_v, rbs, p_os)
                            done_v += 1
                        lastb = (1024 * done_v - 1) // NBCOLS if done_v else 0
                        for k in [k for k in rbs if k < lastb]:
                            del rbs[k]
    nc.finalize()
    return nc


def _host_prep(edge_index, Y, t_normalized, emb, tW1, tW2, projW,
               msgW1, msgW2, updW, eW1, eW2, eW3):
    bf = ml_dtypes.bfloat16
    f8 = ml_dtypes.float8_e4m3
    ei = np.asarray(edge_index)
    ar = np.arange(N, dtype=ei.dtype)
    src = np.concatenate([ei[0], ar])
    dst = np.concatenate([ei[1], ar])
    adj = np.zeros((N, N), np.float32)          # adj[src, dst] edge counts
    np.add.at(adj, (src, dst), 1.0)
    deg = adj.sum(axis=0)                        # in-degree per dst (>=1)
    # pair-blocked DoubleRow layout: [p, a*2048 + half*1024 + c*512 + n]
    adj8 = adj.reshape(4, 2, P, 2, 512).transpose(2, 0, 3, 1, 4).reshape(P, 8 * N)

    h0 = np.asarray(emb, np.float32)[np.asarray(Y)]        # [N, H] gather
    wf32 = np.zeros((P, WF32_C), np.float32)
    wf32[:, 0] = np.float32(np.asarray(t_normalized)[0])
    wf32[:, 1:2] = np.asarray(tW1, np.float32).T           # [H,1]
    wf32[:, 2:2 + P] = np.asarray(tW2, np.float32) @ np.asarray(projW, np.float32)

    updW = np.asarray(updW, np.float32)
    msgW2 = np.asarray(msgW2, np.float32)
    w2u = np.einsum("lij,ljk->lik", msgW2, updW[:, H:2 * H])   # [L, H, H]
    wbf = np.zeros((P, WBF_COLS), np.float32)
    for l in range(L):
        wbf[:, l * WL:l * WL + H] = np.asarray(msgW1)[l]
        wbf[:, l * WL + H:l * WL + 2 * H] = updW[l, 0:H]
        wbf[:, l * WL + 2 * H:l * WL + 3 * H] = w2u[l]
    wbf[:, E1TO:E1TO + H] = np.asarray(eW1)[:H]
    wbf[:, E1BO:E1BO + H] = np.asarray(eW1)[H:]
    wbf[:, E2O:E2O + H] = np.asarray(eW2)
    wbf[:, E3O:E3O + 2] = np.asarray(eW3)

    main = np.zeros((P, MAIN_COLS), np.float32)
    main[:, WF32_O:WF32_O + WF32_C] = wf32
    main[:, PROJ_O:H0T_O] = (
        np.asarray(projW, np.float32).astype(bf).view(np.float32))
    main[:, H0T_O:WBF_O] = h0.T.copy().astype(bf).view(np.float32)
    main[:, WBF_O:MAIN_COLS] = wbf.astype(bf).view(np.float32)

    m = {
        "main": main,
        "adj8": adj8.astype(f8),
        "rdeg": (1.0 / deg)[None, :].astype(np.float32),
    }
    return [dict(m) for _ in range(NCORES)]


def _slot_maps():
    """(pair_index, gather_index) per (core k, vr t, slot s)."""
    k = np.arange(NCORES)[:, None, None]
    t = np.arange(VR)[None, :, None]
    s = np.arange(1024)[None, None, :]
    s1k = 1023 - 8 * t - k
    fwd = s < s1k
    i = np.where(fwd, 8 * t + k, 1022 - 8 * t - k)
    j = np.where(fwd, 8 * t + k + 1 + s, 2046 - 8 * t - k - s)
    idx = i * 1023 - (i * (i - 1)) // 2 + (j - i - 1)
    # device col for (t, s): b=t//32, w=t%32, c=s//128, p=s%128, o in {0,1}
    b, w = t // 32, t % 32
    c, p = s // 128, s % 128
    col = 512 * b + 16 * w + 2 * c
    gidx = p * 1024 + col          # into dev[k].reshape(-1) (row-major [128,1024])
    return idx, np.broadcast_to(gidx, idx.shape).copy()


def timeline_ns():
    if "nc" not in _CACHE:
        _CACHE["nc"] = _build_nc()
        _CACHE["maps"] = _slot_maps()
    from concourse.timeline_sim import TimelineSim
    return TimelineSim(_CACHE["nc"]).simulate()


def kernel(**inputs) -> np.ndarray:
    global LAST_RESULTS
    if "nc" not in _CACHE:
        _CACHE["nc"] = _build_nc()
        _CACHE["maps"] = _slot_maps()
    nc = _CACHE["nc"]
    in_maps = _host_prep(**inputs)
    res = run_bass_kernel_spmd(nc, in_maps, core_ids=list(range(NCORES)),
                               trace=TRACE)
    LAST_RESULTS = res
    idx, gidx = _CACHE["maps"]
    out = np.empty((N * (N - 1) // 2, 2), np.float32)
    for k in range(NCORES):
        dev = res.results[k]["logits_v"].reshape(-1)
        out[idx[k], 0] = dev[gidx[k]]
        out[idx[k], 1] = dev[gidx[k] + 1]
    return out


if __name__ == "__main__":
    sys.path.insert(0, "/root/problem")
    import jax
    with jax.default_device(jax.devices("cpu")[0]):
        import reference
        inp = {k: np.asarray(v) for k, v in reference.setup_inputs().items()}
        exp = np.asarray(reference.reference(**reference.setup_inputs()))
    got = kernel(**inp)
    scale = np.abs(exp).max()
    err = np.abs(got - exp).max() / scale
    print("max abs:", np.abs(got - exp).max(), "scale:", scale, "rel:", err)


# revision 73
# speedup vs baseline: 1.0019x; 1.0019x over previous
"""Trainium2 Bass kernel for BiasFreeDenoisingGNN (N=1024, H=128, E=32768, L=3).

Strategy (8 NeuronCores, one SPMD program, NO collectives):
  - Message passing is fully REPLICATED on every core (collectives cost
    >=15us each, so sharded+AllGather designs lose).  The segment-sum is
    aggT = sum_c relu_r1_c^T @ adj_c with adjacency edge-counts in fp8e4
    (exact small ints, DoubleRow perf mode); the mean-division is fused
    into the PSUM->SBUF copy as a multiply with a broadcast 1/deg row
    (broadcast on-device via gpsimd partition_broadcast from a [1,N] DMA).
  - All f32/bf16 inputs ride in ONE packed f32 dram tensor ("main") with
    bf16 regions read through bitcast views; DMAs are split in need-order
    (h0/weights first, adjacency quarters, remaining weights last) to
    minimize the per-DMA HWDGE/SEQ ladder (each DMA adds a fixed ~1.3us
    issue-latency step).
  - PSUM tiles are split per-half everywhere two engines evict or a write
    follows a disjoint read: the tile dep-tracker serializes cross-engine
    accesses of one PSUM tile (~0.6us per occurrence); p_h, p_r1, p_up,
    p_a, p_b are all split.  HW NOTE: emitting a chunk's basis and rlp
    matmuls as two separate passes corrupts results on hardware (sim is
    fine) - keep each chunk's start/stop accumulation pair adjacent.
  - Edge predictor: 512 virtual rows of 1024 pairs across 8 cores, rows
    v=8t+k fused with 1022-v so each VR is one dense 1024-slot span.
    X = relu(A_i + B_j) built by Pool(POOL_X-col static slice)+DVE
    tensor_scalars at DVE 4x; eW2 on PE; the relu PSUM->SBUF transport is
    batched over 1.5-VR (1536-col) batches into an Act-evicted [P,1024]
    tile plus a DVE-evicted [P,512] tile (separate tags -> true double
    buffering, one reader per PSUM tile, amortized access penalties);
    eW3 = 8 tiny 2-col matmuls per VR into persistent PSUM staging copied
    out by Act once per 16 VRs.  Tiny keep-warm matmuls tied to prep
    outputs hold the PE p-state up through the MP->predictor gap.
"""
import sys
import numpy as np

sys.path.insert(0, "/opt/trn_rl_repo")

import concourse.bass as bass  # noqa: E402
import concourse.bacc as bacc  # noqa: E402
import concourse.mybir as mybir  # noqa: E402
import concourse.tile as tile  # noqa: E402
from concourse.bass_utils import run_bass_kernel_spmd  # noqa: E402
import ml_dtypes  # noqa: E402

N = 1024
H = 128
E = 32768
L = 3
C = 10
NCORES = 8
P = 128
VR = 64             # virtual rows per core
DT = mybir.dt
F32 = DT.float32
F32R = DT.float32r
BF16 = DT.bfloat16
FP8 = DT.float8e4
AF = mybir.ActivationFunctionType
OP = mybir.AluOpType

# ---- packed "main" f32 input layout (f32 cols) ----
# [wf32 130 | proj(bf16) 64 | h0t(bf16) 512 | wbf(bf16) 769]
WF32_O, WF32_C = 0, 130
PROJ_O = 130            # bf16 [128, 128] as 64 f32 cols
H0T_O = 194             # bf16 [128, 1024] as 512 f32 cols
WBF_O = 706             # bf16 [128, 1538] as 769 f32 cols
MAIN_COLS = 1475
# bf16 col offsets inside the wbf view (layer-major)
WL = 3 * H              # per-layer stride: m1 | upd | w2u
E1TO, E1BO, E2O, E3O = 3 * WL, 3 * WL + H, 3 * WL + 2 * H, 3 * WL + 3 * H
WBF_COLS = 3 * WL + 3 * H + 2
# DMA split points (f32 cols of main): D0 = [0:770] (wf32..m1L0),
# D3 = [770:898] (updL0,w2uL0), D5 = [898:1475] (L1,L2,e-weights)
D0_END = WBF_O + (WL + H) // 2      # through m1L0: 706 + 64 = 770... (m1L0 is wbf[0:128])
D0_END = WBF_O + H // 2 * 1         # 706 + 64 = 770
D3_END = WBF_O + (H + 2 * H) // 2   # through w2uL0: 706 + 192 = 898

import os as _os
POOL_X = int(_os.environ.get("KPOOLX", 426))   # Pool's fixed X-build slice per VR
DVE_R = int(_os.environ.get("KDVER", 328))     # DVE's relu slice per VR

_CACHE = {}
LAST_RESULTS = None
TRACE = False


def _build_nc(phases="all"):
    import os
    phases = os.environ.get("KPHASES", phases)
    nc = bacc.Bacc("TRN2", target_bir_lowering=False, debug=False,
                   enable_asserts=True, num_devices=NCORES)
    # --- kernel I/O ---
    main_d = nc.dram_tensor("main", [P, MAIN_COLS], F32, kind="ExternalInput")
    adj_d = nc.dram_tensor("adj8", [P, 8 * N], FP8, kind="ExternalInput")
    rdeg_d = nc.dram_tensor("rdeg", [1, N], F32, kind="ExternalInput")
    logits_d = nc.dram_tensor("logits_v", [P, 1024], F32, kind="ExternalOutput")
    debug = os.environ.get("KDEBUG") == "1"
    if debug:
        dbg_rsc = nc.dram_tensor("dbg_rsc", [P, N], BF16, kind="ExternalOutput")
        dbg_rls = nc.dram_tensor("dbg_rls", [P, N], BF16, kind="ExternalOutput")
        dbg_rdg = nc.dram_tensor("dbg_rdg", [P, N], F32, kind="ExternalOutput")
        dbg_r1 = nc.dram_tensor("dbg_r1", [P, N], BF16, kind="ExternalOutput")
        dbg_h1 = nc.dram_tensor("dbg_h1", [P, N], BF16, kind="ExternalOutput")
        dbg_hf = nc.dram_tensor("dbg_hf", [P, N], F32, kind="ExternalOutput")
        dbg_at = nc.dram_tensor("dbg_at", [P, N], F32, kind="ExternalOutput")
        dbg_bx = nc.dram_tensor("dbg_bx", [P, 2 * N], BF16, kind="ExternalOutput")
        dbg_xb = nc.dram_tensor("dbg_xb", [P, 1032], BF16, kind="ExternalOutput")
        dbg_rb = nc.dram_tensor("dbg_rb", [P, N], BF16, kind="ExternalOutput")

    with tile.TileContext(nc) as tc:
        with tc.tile_pool(name="cst", bufs=1) as cst, \
             tc.tile_pool(name="wk", bufs=3) as wk, \
             tc.tile_pool(name="xp", bufs=4) as xp, \
             tc.tile_pool(name="rp", bufs=5) as rp, \
             tc.tile_pool(name="ps", bufs=2, space="PSUM") as ps:

            kreg = nc.partition_id()

            # ---- constant loads (order = need order) ----
            main = cst.tile([P, MAIN_COLS], F32)
            adj8 = cst.tile([P, 8 * N], FP8)
            rdeg1 = cst.tile([1, N], F32)
            rdegb = cst.tile([P, N], F32)
            nc.sync.dma_start(main[:, 0:D0_END], main_d[:, 0:D0_END])
            nc.sync.dma_start(rdeg1[:], rdeg_d[:])
            nc.sync.dma_start(adj8[:, 0:2 * N], adj_d[:, 0:2 * N])
            nc.sync.dma_start(adj8[:, 2 * N:4 * N], adj_d[:, 2 * N:4 * N])
            nc.sync.dma_start(adj8[:, 4 * N:6 * N], adj_d[:, 4 * N:6 * N])
            nc.sync.dma_start(adj8[:, 6 * N:8 * N], adj_d[:, 6 * N:8 * N])
            nc.sync.dma_start(main[:, D0_END:D3_END], main_d[:, D0_END:D3_END])
            nc.sync.dma_start(main[:, D3_END:MAIN_COLS],
                              main_d[:, D3_END:MAIN_COLS])
            nc.gpsimd.partition_broadcast(rdegb[:], rdeg1[:])
            wf32 = main[:, WF32_O:WF32_O + WF32_C]
            proj_t = main[:, PROJ_O:H0T_O].bitcast(BF16)
            h0t = main[:, H0T_O:WBF_O].bitcast(BF16)
            wbf = main[:, WBF_O:MAIN_COLS].bitcast(BF16)

            # warm-ups while DMAs run: hoist the 1.3us ACT_TABLE_LOAD off the
            # h0 chain, and start the PE p-state ramp clock early with a tiny
            # matmul on memset data; the ones row also broadcasts rdeg via PE
            warm = cst.tile([P, 1], BF16)
            nc.vector.memset(warm[:], 0.0)
            warm2 = cst.tile([P, 1], F32)
            nc.scalar.activation(warm2[:], warm[:], AF.Relu)
            p_warm = ps.tile([P, 1], F32, space="PSUM", tag="half")
            nc.tensor.matmul(out=p_warm[0:1, 0:1], lhsT=warm[:, 0:1],
                             rhs=warm[:, 0:1], start=True, stop=True)

            t_rep = wf32[:, 0:1]
            tw1t = wf32[:, 1:2]
            w2p = wf32[:, 2:2 + P]

            # ---- h0: x1 = relu(t * tW1^T); t2 = (tW2@projW)^T @ x1 ----
            x1t = cst.tile([H, 1], F32)
            nc.vector.tensor_scalar(out=x1t[:], in0=tw1t, scalar1=t_rep,
                                    scalar2=0.0, op0=OP.mult, op1=OP.max)
            p_t2 = ps.tile([P, 1], F32, space="PSUM", tag="half")
            nc.tensor.matmul(out=p_t2[:], lhsT=w2p, rhs=x1t[:],
                             start=True, stop=True)
            t2 = cst.tile([P, 1], F32)
            nc.vector.tensor_copy(t2[:], p_t2[:])

            # h1 = relu(h0 @ projW + t2)  (t2 per-partition bias).  Per-half
            # PSUM tiles (coarse PSUM dep-tracking would serialize the second
            # mm behind the first half's relu otherwise); h1 lands in f32 as
            # h32[0] and hbf[0] is a cheap SBUF->SBUF downcast on DVE.
            h32 = [cst.tile([P, N], F32, name=f"h32_{i}") for i in range(2)]
            hbf = [cst.tile([P, N], BF16, name=f"hbf_{i}") for i in range(2)]
            p_hs = [ps.tile([P, 512], F32, space="PSUM", tag="half",
                            name=f"p_h{hh}") for hh in range(2)]
            for half in range(2):
                sl = slice(half * 512, (half + 1) * 512)
                nc.tensor.matmul(out=p_hs[half][:], lhsT=proj_t,
                                 rhs=h0t[:, sl], start=True, stop=True)
                if half == 0:
                    nc.vector.tensor_scalar(out=h32[0][:, sl], in0=p_hs[0][:],
                                            scalar1=t2[:, :1], scalar2=0.0,
                                            op0=OP.add, op1=OP.max)
                else:
                    nc.scalar.activation(h32[0][:, sl], p_hs[1][:], AF.Relu,
                                         bias=t2[:, :1])
                nc.vector.tensor_copy(hbf[0][:, sl], h32[0][:, sl])
            emit_h32_0 = []
            if debug:
                nc.sync.dma_start(dbg_h1[:], hbf[0][:])

            # ---- message passing (replicated, no comms) ----
            # msgW2 is host-folded into the update weights (W2U = msgW2 @
            # updW_bot), so each layer is: r1 rows -> relu -> adjacency
            # contraction -> deg-scale -> update.  Layer l>=1 reads h via the
            # linear basis (h_{l-1}, rl_{l-1}) so the residual adds run off
            # the critical path.
            rl_prev = None
            for l in (range(L) if phases in ("all", "mp") else []):
                cur, nxt = l % 2, (l + 1) % 2
                m1l = wbf[:, l * WL:l * WL + H]
                updl = wbf[:, l * WL + H:l * WL + 2 * H]
                w2ul = wbf[:, l * WL + 2 * H:l * WL + 3 * H]

                # r1 rows: chunk c -> [node, h1]; two per-half PSUM tiles so
                # the Act/DVE relu halves don't get a false cross-reader dep
                p_r1s = [ps.tile([P, 512], F32, space="PSUM", tag="half",
                                 name=f"p_r1{l}{hh}") for hh in range(2)]
                for c in range(8):
                    osl = slice(c * P, (c + 1) * P)
                    dst = p_r1s[c // 4][:, (c % 4) * P:(c % 4 + 1) * P]
                    if l == 0:
                        nc.tensor.matmul(out=dst, lhsT=hbf[cur][:, osl],
                                         rhs=m1l, start=True, stop=True)
                    else:
                        rlp = rl_prev[c // 4][:, (c % 4) * P:(c % 4 + 1) * P]
                        nc.tensor.matmul(out=dst, lhsT=hbf[nxt][:, osl],
                                         rhs=m1l, start=True, stop=False)
                        nc.tensor.matmul(out=dst, lhsT=rlp,
                                         rhs=m1l, start=False, stop=True)
                # two separate dest tiles (Act/DVE), fewer dependency hops
                r1rh = [wk.tile([P, 512], FP8, tag=f"r1h{qq}",
                                name=f"r1r{l}{qq}") for qq in range(2)]
                nc.scalar.activation(r1rh[0][:], p_r1s[0][:], AF.Relu)
                nc.vector.tensor_scalar(out=r1rh[1][:], in0=p_r1s[1][:],
                                        scalar1=0.0, scalar2=None, op0=OP.max)
                # deferred off-chain work from the previous layer (runs in
                # this layer's agg window, keeps it off DVE's critical path).
                # MUST be emitted before the upd-base matmuls below, which
                # read the hbf[cur] these adds produce; natural priority so
                # the scheduler keeps them AFTER this layer's r1 relus.
                for fn in emit_h32_0:
                    fn()
                emit_h32_0 = []
                # upd "base" matmuls don't depend on agg: emit early so PE
                # fills the relu/agg wait gaps.  Per-half tiles: coarse PSUM
                # dep-tracking would serialize the stop-mm of half 1 behind
                # the rls read of half 0 otherwise.
                p_ups = [ps.tile([P, 512], F32, space="PSUM", tag="big",
                                 bufs=2, name=f"p_up{l}{hh}") for hh in range(2)]
                for half in range(2):
                    sl = slice(half * 512, (half + 1) * 512)
                    nc.tensor.matmul(out=p_ups[half][:], lhsT=updl,
                                     rhs=hbf[cur][:, sl], start=True, stop=False)

                # ragg[h1, d] = sum_c relu_r1_c^T @ adj_c  (h0/h1 col-halves in
                # separate PSUM tiles; a-outer order so the a={0,1} matmuls
                # need only the Act relu half + the first adjacency DMA)
                p_aggs = [ps.tile([P, 512], F32, space="PSUM", tag="half",
                                  name=f"p_agg{l}{hh}") for hh in range(2)]
                # emission order: all of p_agg0's a-steps relu0-first, and
                # p_agg1's group STARTS at a=2 (needs the DVE relu anyway) so
                # its buffer-WAR wait on p_r1s[1] doesn't block the PE queue
                agg_order = [(0, 0), (1, 0), (2, 0), (3, 0),
                             (2, 1), (3, 1), (0, 1), (1, 1)]
                agg_start = {(0, 0): True, (2, 1): True}
                agg_stop = {(3, 0): True, (1, 1): True}
                for a, half in agg_order:
                    nc.tensor.matmul(
                        out=p_aggs[half][:],
                        lhsT=r1rh[a // 2][:, (a % 2) * 256:(a % 2 + 1) * 256]
                            .rearrange("p (c h) -> p c h", c=2),
                        rhs=adj8[:, 2048 * a + 1024 * half:
                                 2048 * a + 1024 * half + 1024]
                            .rearrange("p (c n) -> p c n", c=2),
                        start=agg_start.get((a, half), False),
                        stop=agg_stop.get((a, half), False),
                        perf_mode=mybir.MatmulPerfMode.DoubleRow)
                rscs = [wk.tile([P, 512], BF16, tag=f"agg{hh}",
                                name=f"rsc{l}{hh}") for hh in range(2)]
                rls = [wk.tile([P, 512], BF16, tag=f"rl{hh}",
                               name=f"rl{l}{hh}") for hh in range(2)]
                for half in range(2):
                    sl = slice(half * 512, (half + 1) * 512)
                    nc.vector.tensor_tensor(out=rscs[half][:], in0=p_aggs[half][:],
                                            in1=rdegb[:, sl], op=OP.mult)
                for half in range(2):
                    nc.tensor.matmul(out=p_ups[half][:], lhsT=w2ul,
                                     rhs=rscs[half][:], start=False, stop=True)
                    nc.scalar.activation(rls[half][:], p_ups[half][:], AF.Relu)
                if debug and l == 1:
                    nc.sync.dma_start(dbg_rsc[:, 0:512], rscs[0][:])
                    nc.sync.dma_start(dbg_rsc[:, 512:N], rscs[1][:])
                    nc.sync.dma_start(dbg_rls[:, 0:512], rls[0][:])
                    nc.sync.dma_start(dbg_rls[:, 512:N], rls[1][:])
                    nc.sync.dma_start(dbg_rdg[:], rdegb[:])
                    r1dbg = cst.tile([P, N], BF16)
                    nc.vector.tensor_copy(r1dbg[:, 0:512], r1rh[0][:])
                    nc.vector.tensor_copy(r1dbg[:, 512:N], r1rh[1][:])
                    nc.sync.dma_start(dbg_r1[:], r1dbg[:])
                if l < L - 1:  # residual adds, deferred into next layer's body
                    def mk_adds(rl_t, cu, nx):
                        def emit():
                            for half in range(2):
                                sl = slice(half * 512, (half + 1) * 512)
                                nc.vector.tensor_tensor(out=hbf[nx][:, sl],
                                                        in0=rl_t[half][:],
                                                        in1=h32[cu][:, sl],
                                                        op=OP.add)
                                nc.gpsimd.tensor_tensor(out=h32[nx][:, sl],
                                                        in0=rl_t[half][:],
                                                        in1=h32[cu][:, sl],
                                                        op=OP.add)
                        return emit
                    emit_h32_0 = [mk_adds(rls, cur, nxt)]
                rl_prev = rls

            # ---- predictor prep: A^T (f32), bext = [B^T, reversed B^T] ----
            # final h = hbf[cur-of-last-layer] + rl_prev (linear basis)
            fcur = (L - 1) % 2
            e1t = wbf[:, E1TO:E1TO + H]
            e1b = wbf[:, E1BO:E1BO + H]
            e2 = wbf[:, E2O:E2O + H]
            e3 = wbf[:, E3O:E3O + 2]
            if phases == "pred":
                fdelta = [hbf[0][:, 0:512], hbf[0][:, 512:N]]
            else:
                fdelta = [rl_prev[0][:], rl_prev[1][:]]
            fbase = hbf[fcur]
            # per-half PSUM tiles for A/B so the Act and DVE evictions of
            # the two halves don't pick up a false cross-reader dependency
            p_as = [ps.tile([P, 512], F32, space="PSUM", tag="dj", bufs=2,
                            name=f"p_a{hh}") for hh in range(2)]
            for half in range(2):
                sl = slice(half * 512, (half + 1) * 512)
                nc.tensor.matmul(out=p_as[half][:], lhsT=e1t, rhs=fbase[:, sl],
                                 start=True, stop=False)
                nc.tensor.matmul(out=p_as[half][:], lhsT=e1t, rhs=fdelta[half],
                                 start=False, stop=True)
            AT_f = cst.tile([P, N], F32)
            nc.scalar.activation(AT_f[:, 0:512], p_as[0][:], AF.Copy)
            nc.vector.tensor_copy(AT_f[:, 512:N], p_as[1][:])
            p_bs = [ps.tile([P, 512], F32, space="PSUM", tag="big", bufs=2,
                            name=f"p_b{hh}") for hh in range(2)]
            for half in range(2):
                sl = slice(half * 512, (half + 1) * 512)
                nc.tensor.matmul(out=p_bs[half][:], lhsT=e1b, rhs=fbase[:, sl],
                                 start=True, stop=False)
                nc.tensor.matmul(out=p_bs[half][:], lhsT=e1b, rhs=fdelta[half],
                                 start=False, stop=True)
            bext = cst.tile([P, 2 * N], BF16)
            nc.vector.tensor_copy(bext[:, 0:512], p_bs[0][:])
            nc.scalar.activation(bext[:, 512:N], p_bs[1][:], AF.Copy)
            # only rev cols [N, N+528) are ever read (wrap s2<=512, bsh tail);
            # build them with a 4x SBUF->SBUF reversed copy instead of a
            # full-width PSUM eviction
            nc.vector.tensor_copy(bext[:, N:N + 528],
                                  bext[:, 496:N][:, ::-1])
            # per-core shifted copies so the Pool X-build ops use fully
            # static access patterns (gpsimd mishandles register-offset APs);
            # ash + first bsh half unblock VR0's Pool op earliest
            ash = cst.tile([P, 512], F32)
            nc.scalar.activation(ash[:], AT_f[:, bass.ds(kreg, 512)], AF.Copy)
            # PE p-state keep-warm: tiny matmuls tied to prep outputs spread
            # across the otherwise-idle PE window so the first eW2 matmuls
            # don't pay the cold-ramp penalty
            for wi, wsrc in enumerate((bext[:, 0:1], bext[:, 700:701],
                                       bext[:, N:N + 1])):
                p_w = ps.tile([P, 1], F32, space="PSUM", tag="half",
                              name=f"p_wu{wi}")
                nc.tensor.matmul(out=p_w[0:1, 0:1], lhsT=wsrc,
                                 rhs=warm[:, 0:1], start=True, stop=True)
            bsh = cst.tile([P, 1032], BF16)
            nc.vector.tensor_copy(bsh[:, 0:516], bext[:, bass.ds(kreg, 516)])
            nc.vector.tensor_copy(bsh[:, 516:1032],
                                  bext[:, bass.ds(kreg + 516, 516)])
            if debug:
                nc.sync.dma_start(dbg_at[:], AT_f[:])
                nc.sync.dma_start(dbg_bx[:], bext[:])
                hfd = cst.tile([P, N], F32)
                for _h in range(2):
                    _sl = slice(_h * 512, (_h + 1) * 512)
                    nc.vector.tensor_tensor(out=hfd[:, _sl], in0=fbase[:, _sl],
                                            in1=fdelta[_h], op=OP.add)
                nc.sync.dma_start(dbg_hf[:], hfd[:])

            # ---- predictor: 64 virtual rows ----
            def emit_x(t):
                s1 = 1023 - 8 * t
                s2 = 8 * t + 8
                xb = xp.tile([P, 1032], BF16, tag="x")
                if t < 2:
                    # first VRs: DVE-only dynamic APs (proven on HW), so the
                    # Pool shift-prep (bsh/ash) finishes in their shadow
                    a_dyn = AT_f[:, bass.ds(kreg + 8 * t, 1)]
                    nc.vector.tensor_scalar(
                        out=xb[:, 0:s1],
                        in0=bext[:, bass.ds(kreg + (8 * t + 1), s1)],
                        scalar1=a_dyn, scalar2=0.0, op0=OP.add, op1=OP.max)
                else:
                    a_fwd = ash[:, 8 * t:8 * t + 1]
                    nc.gpsimd.tensor_scalar(
                        out=xb[:, 0:POOL_X],
                        in0=bsh[:, 8 * t + 1:8 * t + 1 + POOL_X],
                        scalar1=a_fwd, scalar2=0.0, op0=OP.add, op1=OP.max)
                    nc.vector.tensor_scalar(
                        out=xb[:, POOL_X:s1],
                        in0=bsh[:, 8 * t + 1 + POOL_X:8 * t + 1 + s1],
                        scalar1=a_fwd, scalar2=0.0, op0=OP.add, op1=OP.max)
                nc.vector.tensor_scalar(
                    out=xb[:, bass.ds(s1 - kreg, s2)],
                    in0=bext[:, N:N + s2],
                    scalar1=AT_f[:, bass.ds((1022 - 8 * t) - kreg, 1)],
                    scalar2=0.0, op0=OP.add, op1=OP.max)
                return xb

            # relu evictions are batched over 1.5-VR (1536-col) batches:
            # each batch owns an Act-evicted [P,1024] tile and a DVE-evicted
            # [P,512] tile (separate tags -> true double buffering, and one
            # reader per PSUM tile so no cross-engine reader serialization).
            # Amortizes the per-op PSUM access penalty over 1536 cols.
            NBCOLS = 1536
            TOT = VR * 1024
            NB = (TOT + NBCOLS - 1) // NBCOLS

            def emit_batch_mms(b, xbs):
                base = b * NBCOLS
                cols = min(NBCOLS, TOT - base)
                pa = ps.tile([P, 1024], F32, space="PSUM", tag="big",
                             bufs=2, name=f"pa{b}")
                pd = (ps.tile([P, 512], F32, space="PSUM", tag="dj",
                              bufs=2, name=f"pd{b}") if cols > 1024 else None)
                for c in range(0, cols, 512):
                    gc = base + c
                    v, half = gc // 1024, (gc % 1024) // 512
                    if v not in xbs:
                        xbs[v] = emit_x(v)
                        if debug and v == 0:
                            nc.sync.dma_start(dbg_xb[:], xbs[v][:])
                    tile_, off = (pa, c) if c < 1024 else (pd, c - 1024)
                    nc.tensor.matmul(out=tile_[:, off:off + 512], lhsT=e2,
                                     rhs=xbs[v][:, half * 512:half * 512 + 512],
                                     start=True, stop=True)
                for v in [k for k in xbs if (k + 1) * 1024 <= base + cols]:
                    del xbs[v]
                return (pa, pd, cols)

            def emit_batch_relu(b, pys_b):
                pa, pd, cols = pys_b
                ca = min(cols, 1024)
                rba = rp.tile([P, 1024], BF16, tag="ra", bufs=4,
                              name=f"rba{b}")
                nc.scalar.activation(rba[:, 0:ca], pa[:, 0:ca], AF.Relu)
                rbd = None
                if pd is not None:
                    rbd = rp.tile([P, 512], BF16, tag="rd", bufs=4,
                                  name=f"rbd{b}")
                    nc.vector.tensor_scalar(out=rbd[:, 0:cols - 1024],
                                            in0=pd[:, 0:cols - 1024],
                                            scalar1=0.0, scalar2=None,
                                            op0=OP.max)
                if debug and b == 0:
                    nc.sync.dma_start(dbg_rb[:], rba[:, 0:N])
                return (rba, rbd)

            def emit_ew3_v(v, rbs, p_os):
                blk = v // 32
                if v % 32 == 0:
                    p_os[blk] = ps.tile([P, 512], F32, space="PSUM",
                                        tag="half", name=f"p_o{blk}")
                w = v % 32
                for c in range(8):
                    gc = 1024 * v + c * P
                    bb, off = gc // NBCOLS, gc % NBCOLS
                    rba, rbd = rbs[bb]
                    rb_, o2 = (rba, off) if off < 1024 else (rbd, off - 1024)
                    nc.tensor.matmul(
                        out=p_os[blk][:, 16 * w + 2 * c:16 * w + 2 * c + 2],
                        lhsT=rb_[:, o2:o2 + P], rhs=e3,
                        start=True, stop=True)
                # consolidated flushes: fewer, larger Act copies (the 185ns
                # per-op access penalty dominates small pieces); only the
                # final 128-col piece is exposed in the kernel tail
                if v == 15:
                    stg = wk.tile([P, 512], F32, tag="stg", name="stg0")
                    p_os[2] = stg
                    nc.scalar.activation(stg[:, 0:256], p_os[0][:, 0:256],
                                         AF.Copy)
                    nc.sync.dma_start(logits_d[:, 0:256], stg[:, 0:256])
                if v == 31:
                    stg = p_os[2]
                    nc.scalar.activation(stg[:, 256:512], p_os[0][:, 256:512],
                                         AF.Copy)
                    nc.sync.dma_start(logits_d[:, 256:512], stg[:, 256:512])
                if v == 55:
                    stg = wk.tile([P, 512], F32, tag="stg", name="stg1")
                    p_os[3] = stg
                    nc.scalar.activation(stg[:, 0:384], p_os[1][:, 0:384],
                                         AF.Copy)
                    nc.sync.dma_start(logits_d[:, 512:896], stg[:, 0:384])
                if v == 63:
                    stg = p_os[3]
                    nc.scalar.activation(stg[:, 384:512], p_os[1][:, 384:512],
                                         AF.Copy)
                    nc.sync.dma_start(logits_d[:, 896:1024], stg[:, 384:512])

            if phases in ("all", "pred"):
                xbs = {}
                pys = {}
                rbs = {}
                p_os = {}
                done_v = 0
                for step in range(NB + 2):
                    if step < NB:
                        pys[step] = emit_batch_mms(step, xbs)
                    if 1 <= step <= NB:
                        rbs[step - 1] = emit_batch_relu(step - 1,
                                                        pys.pop(step - 1))
                    if step >= 2:
                        # all VRs fully covered by relu'd batches [0, step-1)
                        avail = ((step - 1) * NBCOLS) // 1024
                        while done_v < min(avail, VR):
                            emit_ew3_v(done_v, rbs, p_os)
                            done_v += 1
                        lastb = (1024 * done_v - 1) // NBCOLS if done_v else 0
                        for k in [k for k in rbs if k < lastb]:
                            del rbs[k]
    nc.finalize()
    return nc


def _host_prep(edge_index, Y, t_normalized, emb, tW1, tW2, projW,
               msgW1, msgW2, updW, eW1, eW2, eW3):
    bf = ml_dtypes.bfloat16
    f8 = ml_dtypes.float8_e4m3
    ei = np.asarray(edge_index)
    ar = np.arange(N, dtype=ei.dtype)
    src = np.concatenate([ei[0], ar])
    dst = np.concatenate([ei[1], ar])
    adj = np.zeros((N, N), np.float32)          # adj[src, dst] edge counts
    np.add.at(adj, (src, dst), 1.0)
    deg = adj.sum(axis=0)                        # in-degree per dst (>=1)
    # pair-blocked DoubleRow layout: [p, a*2048 + half*1024 + c*512 + n]
    adj8 = adj.reshape(4, 2, P, 2, 512).transpose(2, 0, 3, 1, 4).reshape(P, 8 * N)

    h0 = np.asarray(emb, np.float32)[np.asarray(Y)]        # [N, H] gather
    wf32 = np.zeros((P, WF32_C), np.float32)
    wf32[:, 0] = np.float32(np.asarray(t_normalized)[0])
    wf32[:, 1:2] = np.asarray(tW1, np.float32).T           # [H,1]
    wf32[:, 2:2 + P] = np.asarray(tW2, np.float32) @ np.asarray(projW, np.float32)

    updW = np.asarray(updW, np.float32)
    msgW2 = np.asarray(msgW2, np.float32)
    w2u = np.einsum("lij,ljk->lik", msgW2, updW[:, H:2 * H])   # [L, H, H]
    wbf = np.zeros((P, WBF_COLS), np.float32)
    for l in range(L):
        wbf[:, l * WL:l * WL + H] = np.asarray(msgW1)[l]
        wbf[:, l * WL + H:l * WL + 2 * H] = updW[l, 0:H]
        wbf[:, l * WL + 2 * H:l * WL + 3 * H] = w2u[l]
    wbf[:, E1TO:E1TO + H] = np.asarray(eW1)[:H]
    wbf[:, E1BO:E1BO + H] = np.asarray(eW1)[H:]
    wbf[:, E2O:E2O + H] = np.asarray(eW2)
    wbf[:, E3O:E3O + 2] = np.asarray(eW3)

    main = np.zeros((P, MAIN_COLS), np.float32)
    main[:, WF32_O:WF32_O + WF32_C] = wf32
    main[:, PROJ_O:H0T_O] = (
        np.asarray(projW, np.float32).astype(bf).view(np.float32))
    main[:, H0T_O:WBF_O] = h0.T.copy().astype(bf).view(np.float32)
    main[:, WBF_O:MAIN_COLS] = wbf.astype(bf).view(np.float32)

    m = {
        "main": main,
        "adj8": adj8.astype(f8),
        "rdeg": (1.0 / deg)[None, :].astype(np.float32),
    }
    return [dict(m) for _ in range(NCORES)]


def _slot_maps():
    """(pair_index, gather_index) per (core k, vr t, slot s)."""
    k = np.arange(NCORES)[:, None, None]
    t = np.arange(VR)[None, :, None]
    s = np.arange(1024)[None, None, :]
    s1k = 1023 - 8 * t - k
    fwd = s < s1k
    i = np.where(fwd, 8 * t + k, 1022 - 8 * t - k)
    j = np.where(fwd, 8 * t + k + 1 + s, 2046 - 8 * t - k - s)
    idx = i * 1023 - (i * (i - 1)) // 2 + (j - i - 1)
    # device col for (t, s): b=t//32, w=t%32, c=s//128, p=s%128, o in {0,1}
    b, w = t // 32, t % 32
    c, p = s // 128, s % 128
    col = 512 * b + 16 * w + 2 * c
    gidx = p * 1024 + col          # into dev[k].reshape(-1) (row-major [128,1024])
    return idx, np.broadcast_to(gidx, idx.shape).copy()


def timeline_ns():
    if "nc" not in _CACHE:
        _CACHE["nc"] = _build_nc()
        _CACHE["maps"] = _slot_maps()
    from concourse.timeline_sim import TimelineSim
    return TimelineSim(_CACHE["nc"]).simulate()


def kernel(**inputs) -> np.ndarray:
    global LAST_RESULTS
    if "nc" not in _CACHE:
        _CACHE["nc"] = _build_nc()
        _CACHE["maps"] = _slot_maps()
    nc = _CACHE["nc"]
    in_maps = _host_prep(**inputs)
    res = run_bass_kernel_spmd(nc, in_maps, core_ids=list(range(NCORES)),
                               trace=TRACE)
    LAST_RESULTS = res
    idx, gidx = _CACHE["maps"]
    out = np.empty((N * (N - 1) // 2, 2), np.float32)
    for k in range(NCORES):
        dev = res.results[k]["logits_v"].reshape(-1)
        out[idx[k], 0] = dev[gidx[k]]
        out[idx[k], 1] = dev[gidx[k] + 1]
    return out


if __name__ == "__main__":
    sys.path.insert(0, "/root/problem")
    import jax
    with jax.default_device(jax.devices("cpu")[0]):
        import reference
        inp = {k: np.asarray(v) for k, v in reference.setup_inputs().items()}
        exp = np.asarray(reference.reference(**reference.setup_inputs()))
    got = kernel(**inp)
    scale = np.abs(exp).max()
    err = np.abs(got - exp).max() / scale
    print("max abs:", np.abs(got - exp).max(), "scale:", scale, "rel:", err)
